# revision 4
# baseline (speedup 1.0000x reference)
# Binary linear: y[b,s,o] = sum_i x[b,s,i] * sign(W)[o,i]
#
# Strategy (8 NeuronCores, data-parallel over tokens):
#   - Host: flatten x to [32768, 768], shard 8 x [4096, 768], transpose each
#     shard to xT [768, 4096] bf16, tiled per 512-token chunk as
#     [128p, 6ksub, 512t] so each chunk is one contiguous 768KB DMA.
#     Weights are exactly +-1 so only x carries bf16 rounding.
#   - Device (per core): weight-stationary matmuls, all N=512 (one PSUM bank):
#     loop token chunk (512) -> out block (128) -> 6 matmuls accumulating
#     psum[128o, 512t] += w[128i,128o].T @ x[128i,512t] over the 6 k-subtiles.
#     N=512 everywhere keeps the PE at its 213ns/matmul stream floor (the
#     old 512+256 split ran the 256 half LDWEIGHTS-bound at 131ns/256rows).
#     psum -> SBUF bf16 copies alternate DVE/ACT; y stored bf16 (halves
#     store traffic vs f32).
#     DMA: x chunks prefetched FIFO on the sync queue, weights early on the
#     scalar queue, y stores on the vector/scalar queues (separate from x
#     so stores don't queue behind loads).
#   - Host: reassemble [4, 8192, 768] f32 from the bf16 tile layout.

import numpy as np

N_CORES = 8
B, S, D = 4, 8192, 768
T_TOTAL = B * S
T_CORE = T_TOTAL // N_CORES   # 4096 tokens per core
P = 128
NS = 6                        # k-subtiles of 128
OB = D // P                   # 6 out blocks
TC = 8                        # token chunks per core
TW = T_CORE // TC             # 512 tokens per chunk

_cache = {}


def _build():
    import concourse.bacc as bacc
    import concourse.mybir as mybir
    import concourse.tile as tile

    f32 = mybir.dt.float32
    bf16 = mybir.dt.bfloat16

    nc = bacc.Bacc(
        "TRN2",
        target_bir_lowering=False,
        debug=False,
        num_devices=N_CORES,
    )

    xD = nc.dram_tensor("x8", [TC * P, NS * TW], bf16, kind="ExternalInput")
    wD = nc.dram_tensor("w8", [OB * P, NS * P], bf16, kind="ExternalInput")
    yD = nc.dram_tensor("y8", [TC * 2 * P, 3 * TW], bf16, kind="ExternalOutput")

    with tile.TileContext(nc) as tc_:
        with (
            tc_.tile_pool(name="wbuf", bufs=1) as wpool,
            tc_.tile_pool(name="xbuf", bufs=1) as xpool,
            tc_.tile_pool(name="ybuf", bufs=4) as ypool,
            tc_.tile_pool(name="psum", bufs=4, space="PSUM") as psum_pool,
        ):
            # --- PE warmup: dummy matmuls during the framework preamble /
            # first DMAs so the p-state + HAM clock gate are at full rate
            # when the real matmuls start. ---
            wu = xpool.tile([P, 640], bf16, tag="warmup", name="wu")
            nc.vector.memset(wu[:], 0.0)
            wups = psum_pool.tile([P, 512], f32, tag="wups", name="wups", bufs=1)
            for k in range(12):
                nc.tensor.matmul(
                    wups[:], wu[:, :P], wu[:, P:P + 512],
                    start=True, stop=True, skip_group_check=True,
                )
            wu_out = xpool.tile([P, 512], f32, tag="warmup_out", name="wu_out")
            nc.vector.tensor_copy(wu_out[:], wups[:])

            # weights: per-ob tiles on the scalar queue (ahead of any y store)
            wt = []
            for ob in range(OB):
                t = wpool.tile([P, NS * P], bf16, tag=f"w{ob}", name=f"w{ob}")
                nc.scalar.dma_start(t[:], wD[ob * P:(ob + 1) * P, :])
                wt.append(t.rearrange("p (i o) -> p i o", i=NS))

            # x chunks: issued upfront on the sync queue; the queue is FIFO so
            # they stream in order ahead of the compute that consumes them.
            xt = []
            for c in range(TC):
                t = xpool.tile([P, NS * TW], bf16, tag=f"x{c}", name=f"x{c}")
                nc.sync.dma_start(t[:], xD[c * P:(c + 1) * P, :])
                xt.append(t.rearrange("p (s t) -> p s t", s=NS))

            for c in range(TC):
                for half in range(2):
                    yt = ypool.tile([P, 3 * TW], bf16, tag="y3", name=f"y{c}_{half}")
                    y3 = yt.rearrange("p (g t) -> p g t", g=3)
                    yrow = (c * 2 + half) * P
                    for g in range(3):
                        ob = half * 3 + g
                        ps = psum_pool.tile([P, TW], f32, tag="ps", name=f"ps{c}_{ob}")
                        for i in range(NS):
                            nc.tensor.matmul(
                                ps[:],
                                wt[ob][:, i, :],
                                xt[c][:, i, :],
                                start=(i == 0),
                                stop=(i == NS - 1),
                            )
                        if ob % 2 == 0:
                            nc.vector.tensor_copy(y3[:, g, :], ps[:])
                        else:
                            nc.scalar.copy(y3[:, g, :], ps[:])
                        if c == TC - 1:
                            # tail: store each out-block as soon as its copy
                            # lands so the final drain is one small store;
                            # alternate scalar HWDGE / gpsimd SWDGE so the
                            # last stores drain in parallel
                            eng = nc.gpsimd if ob % 2 == 0 else nc.scalar
                            eng.dma_start(
                                yD[yrow:yrow + P, g * TW:(g + 1) * TW],
                                y3[:, g, :],
                            )
                    if c < TC - 1:
                        nc.scalar.dma_start(yD[yrow:yrow + P, :], yt[:])

    nc.compile()
    return nc


def _get_nc():
    if "nc" not in _cache:
        _cache["nc"] = _build()
    return _cache["nc"]


def _prep_inputs(x, weight):
    import ml_dtypes

    bf = ml_dtypes.bfloat16
    x = np.asarray(x, dtype=np.float32).reshape(N_CORES, T_CORE, D)
    w = np.asarray(weight, dtype=np.float32)

    # w8[ob*P+p, i*P+o] = sign(W).T[i*128+p, ob*128+o]; +-1 exact in bf16
    sT = np.sign(w).T
    w8 = np.ascontiguousarray(
        sT.reshape(NS, P, OB, P).transpose(2, 1, 0, 3)
    ).reshape(OB * P, NS * P).astype(bf)

    in_maps = []
    for c in range(N_CORES):
        xc = np.ascontiguousarray(x[c].T).astype(bf)    # [768, 4096] bf16
        x8 = np.ascontiguousarray(
            xc.reshape(NS, P, TC, TW).transpose(2, 1, 0, 3)
        ).reshape(TC * P, NS * TW)                      # (tc, p, s, t)
        in_maps.append({"x8": x8, "w8": w8})
    return in_maps


def _decode_out(res):
    # y8 [(tc, h, p), (g, t)] -> y[tok, o]: tok = tc*TW+t, o = (3h+g)*128+p
    outs = []
    for c in range(N_CORES):
        y8 = np.asarray(res.results[c]["y8"]).reshape(TC, 2, P, 3, TW)
        yc = y8.transpose(0, 4, 1, 3, 2).reshape(T_CORE, D)
        outs.append(yc)
    y = np.concatenate(outs, axis=0).astype(np.float32)
    return y.reshape(B, S, D)


def _install_axon_ntff_hook():
    """The agent image's `antenv` lacks `axon_hooks`; register an equivalent
    module backed by direct ctypes calls into libaxon_pjrt.so so that
    run_bass_kernel_spmd(trace=True) can capture NTFF profiles under axon."""
    import sys

    if "antenv.axon_hooks" in sys.modules:
        return
    import contextlib
    import ctypes
    import types

    so_path = "/opt/axon/libaxon_pjrt.so"
    try:
        lib = ctypes.CDLL(so_path)
    except OSError:
        return
    if not hasattr(lib, "axon_start_nrt_profile"):
        return
    lib.axon_start_nrt_profile.argtypes = [
        ctypes.POINTER(ctypes.c_int64),
        ctypes.c_size_t,
    ]
    lib.axon_start_nrt_profile.restype = ctypes.c_int64
    lib.axon_stop_nrt_profile.argtypes = [ctypes.c_char_p]
    lib.axon_stop_nrt_profile.restype = ctypes.c_int64

    @contextlib.contextmanager
    def _hook(output_dir, device_ids):
        import jax

        jax.devices()
        if device_ids:
            ids = (ctypes.c_int64 * len(device_ids))(*device_ids)
            rc = lib.axon_start_nrt_profile(ids, len(device_ids))
        else:
            rc = lib.axon_start_nrt_profile(None, 0)
        if rc != 0:
            raise RuntimeError(f"axon_start_nrt_profile rc={rc}")
        try:
            yield
        finally:
            n = lib.axon_stop_nrt_profile(str(output_dir).encode())
            print(f"ntff profile: {n} file(s) written to {output_dir}")

    mod = types.ModuleType("antenv.axon_hooks")
    mod.get_axon_ntff_profile_hook = lambda: _hook
    mod.set_axon_ntff_profile_hook = lambda h: None
    sys.modules["antenv.axon_hooks"] = mod


def _run(x, weight, trace=False):
    from concourse.bass_utils import run_bass_kernel_spmd

    if trace:
        _install_axon_ntff_hook()
    nc = _get_nc()
    in_maps = _prep_inputs(x, weight)
    res = run_bass_kernel_spmd(
        nc, in_maps, core_ids=list(range(N_CORES)), trace=trace
    )
    return _decode_out(res), res


def kernel(x, weight):
    out, _ = _run(x, weight, trace=False)
    return out
